# revision 1
# baseline (speedup 1.0000x reference)
"""Trainium2 Bass kernel for the branch-sparse dendritic LIF SNN forward pass.

Self-contained: hardcodes shapes from the problem spec.
  x (256,250,700) f32, target (256,250) int, mem0 (256,512) f32,
  W (1024,700) f32, tau_m (512,) f32, tau_n (512,2) f32,
  W2 (20,512) f32, b2 (20,) f32, mask (1024,700) bool
Returns (loss, correct, total) matching the jax reference.

Strategy: data-parallel over batch across 8 NeuronCores (32 rows each).
Per core:
  - GEMM cur' = wfold^T @ x^T in bf16 (weights pre-folded with (1-a)(1-b),
    branch-major row order). n-blocks hold two batch rows separated by a
    zero column (width 501) so one tensor_tensor_scan with a beta=0
    boundary element covers both rows (IIR state resets through the zero).
  - Dendritic IIR via tensor_tensor_scan along t straight from PSUM
    (fp32 state), branch pairs summed into an li history (bf16) on GPSIMD.
  - Sequential membrane loop over t: 2 custom fused DVE ops per step
    (mem' = li_t - alpha*((mem>1)-mem), alpha selected per sub-page via
    SubIdx), ping-pong mem tiles; spike history recorded as sign(mem-1)
    on the Scalar engine off the critical path.
  - Classifier matmuls on the +-1 spike code with 0.5*W2 bf16 and bias
    folded as b2 + 0.5*sum_h W2 (hi+lo bf16 split via K=1 ones-matmul);
    log-softmax/NLL/argmax head on device, per-core scalars out.
Host combines the 8 per-core scalars.
"""

import dataclasses

import numpy as np
import ml_dtypes

import concourse.bass as bass
import concourse.tile as tile
from concourse import bacc, mybir, bass_utils

F32 = mybir.dt.float32
BF16 = mybir.dt.bfloat16
OP = mybir.AluOpType

B, T, D, H, O, BR = 256, 250, 700, 512, 20, 2
NCORES = 8
BC = B // NCORES            # 32 batch rows per core
DP = 768                    # D padded to 6 k-tiles of 128
KT = DP // 128              # 6
NS = BC * T                 # 8000 samples per core
BLK = 2 * T                 # 500 payload samples per n-block (2 batch rows)
BLKZ = BLK + 1              # 501 with the zero separator column
NB = NS // BLK              # 16 blocks
MT = (H * BR) // 128        # 8 m-tiles (o' = br*512 + h, br-major)
HT = H // 128               # 4 h-tiles
NCH = (NS + 127) // 128     # 63 classifier chunks
NSP = NCH * 128             # 8064 padded samples

_compiled = {}


def _lif_step_op():
    """Fused membrane step: out = Src1 - alpha_sel*((Src0>1) - Src0),
    alpha_sel = C0 on sub-page 0, C1 on sub-page 1."""
    if "op" in _compiled:
        return _compiled["op"]
    from concourse.dve_spec import (
        Spec, Src0, Src1, C0, C1, Zero, One, SubIdx, select, eq, lower)
    from concourse.dve_uop import DveOpSpec
    from concourse import dve_ops

    # Src0 = li_t ([P,2,32] strided, supplies SubIdx); Src1 = mem (rank-1)
    alpha_sel = select(eq(SubIdx, Zero), C0, C1)
    body = Src0 - alpha_sel * ((Src1 > One) - Src1)

    def _ref(in0, in1, s0, s1, imm2=0.0):
        s0 = np.asarray(s0, np.float32).reshape(-1, 1, 1)
        s1 = np.asarray(s1, np.float32).reshape(-1, 1, 1)
        s = np.where(np.arange(in0.shape[1])[None, :, None] == 0, s0, s1)
        m = np.asarray(in1, np.float32).reshape(in0.shape)
        return in0 - s * ((m > 1.0).astype(np.float32) - m)

    spec = Spec(body=body, reference=_ref)
    shas = {}
    for ver in ("v3", "v4"):
        d = DveOpSpec(name="ANT_LIF_STEP", opcode=0, uops=lower(spec, ver=ver),
                      rd1_en=True)
        shas[ver] = d.sha(ver)
    existing = [o for o in dve_ops.OPS if o.name == "ANT_LIF_STEP"]
    if existing:
        op = existing[0]
    else:
        op = dve_ops.DveOp("ANT_LIF_STEP", spec, subdim=True, uops_sha=shas)
        dve_ops.OPS.append(op)
        dve_ops.CUSTOM_DVE_SPECS[op.name] = op.spec
        dve_ops._SUB_OPCODE_FOR_NAME[op.name] = (
            dve_ops._CUSTOM_DVE_ROW_BASE + len(dve_ops.OPS) - 1)
    _compiled["op"] = op
    return op


def _build_nc():
    lif_op = _lif_step_op()
    nc = bacc.Bacc("TRN2", target_bir_lowering=False, debug=False,
                   num_devices=NCORES)

    xT_d = nc.dram_tensor("xT", [DP, NS], BF16, kind="ExternalInput").ap()
    wqT_d = nc.dram_tensor("wqT", [DP, H * BR], BF16, kind="ExternalInput").ap()
    betab_d = nc.dram_tensor("betab", [128, MT * BLKZ], F32, kind="ExternalInput").ap()
    alpha_d = nc.dram_tensor("alpha", [128, HT], F32, kind="ExternalInput").ap()
    m0_d = nc.dram_tensor("m0", [128, 128], F32, kind="ExternalInput").ap()
    w2hi_d = nc.dram_tensor("w2hi", [128, HT * O], BF16, kind="ExternalInput").ap()
    b2p_d = nc.dram_tensor("b2p", [1, 2 * O], BF16, kind="ExternalInput").ap()
    oneh_d = nc.dram_tensor("oneh", [128, NCH * O], BF16, kind="ExternalInput").ap()
    scal_d = nc.dram_tensor("scal", [1, 2], F32, kind="ExternalOutput").ap()

    with tile.TileContext(nc) as tc:
        with tc.tile_pool(name="const", bufs=1) as cp, \
             tc.tile_pool(name="hist", bufs=1) as hp:
            wq = cp.tile([128, KT * H * BR], BF16, tag="wq")
            for kt in range(KT):
                nc.sync.dma_start(wq[:, kt * 1024:(kt + 1) * 1024],
                                  wqT_d[kt * 128:(kt + 1) * 128, :])
            betab = cp.tile([128, MT * BLKZ], F32, tag="betab")
            nc.sync.dma_start(betab[:], betab_d)
            alpha = cp.tile([128, HT], F32, tag="alpha")
            nc.sync.dma_start(alpha[:], alpha_d)
            w2hi = cp.tile([128, HT * O], BF16, tag="w2hi")
            nc.sync.dma_start(w2hi[:], w2hi_d)
            b2p = cp.tile([1, 2 * O], BF16, tag="b2p")
            nc.sync.dma_start(b2p[:], b2p_d)
            onesr = cp.tile([1, 128], BF16, tag="onesr")
            nc.vector.memset(onesr[:], 1.0)
            oneh = cp.tile([128, NCH * O], BF16, tag="oneh")
            nc.sync.dma_start(oneh[:], oneh_d)

            li = hp.tile([128, HT * NS], BF16, tag="li")
            g = hp.tile([128, HT * NSP], BF16, tag="g")
            g4 = g[:].rearrange("p (ho n) -> p ho n", ho=HT)
            nc.vector.memset(g4[:, :, NS:NSP], 0.0)
            logits = hp.tile([128, NCH * O], F32, tag="logits")

            # ---------------- phase 1: GEMM + dendritic scans ----------
            with tc.tile_pool(name="xin", bufs=3) as xp, \
                 tc.tile_pool(name="ps", bufs=8, space="PSUM") as pp, \
                 tc.tile_pool(name="ee", bufs=10) as ep:
                for blk in range(NB):
                    xb = xp.tile([128, KT * BLKZ], BF16, tag="xb")
                    xb3 = xb[:].rearrange("p (kt n) -> p kt n", kt=KT)
                    for kt in range(KT):
                        for b2i in range(2):
                            nc.sync.dma_start(
                                xb3[:, kt, b2i * (T + 1):b2i * (T + 1) + T],
                                xT_d[kt * 128:(kt + 1) * 128,
                                     (2 * blk + b2i) * T:(2 * blk + b2i + 1) * T])
                    # zero separator column in every k-slot
                    nc.vector.memset(xb3[:, :, T:T + 1], 0.0)
                    for m in range(MT):
                        pt = pp.tile([128, BLKZ], F32, tag="ps")
                        for kt in range(KT):
                            nc.tensor.matmul(
                                pt[:],
                                wq[:, kt * 1024 + m * 128: kt * 1024 + (m + 1) * 128],
                                xb3[:, kt, :],
                                start=(kt == 0), stop=(kt == KT - 1))
                        # one dendritic IIR covering both rows (resets at col T)
                        et = ep.tile([128, BLKZ], F32, tag="e")
                        nc.vector.tensor_tensor_scan(
                            et[:], betab[:, m * BLKZ:(m + 1) * BLKZ], pt[:],
                            0.0, OP.mult, OP.add)
                        if m == 0:
                            e_br0 = [None] * HT
                        if m < HT:
                            e_br0[m] = et
                        else:
                            ho = m - HT
                            # rows live at cols [0:T] and [T+1:2T+1] of the
                            # 501-wide e tiles; skip the separator column
                            def _rows(ap):
                                return dataclasses.replace(
                                    ap, ap=[ap.ap[0], [T + 1, 2], [1, T]])
                            li_out = li[:].rearrange(
                                "p (ho n) -> p ho n", ho=HT)[
                                :, ho, blk * BLK:(blk + 1) * BLK].rearrange(
                                "p (b2 t) -> p b2 t", t=T)
                            nc.gpsimd.tensor_add(
                                li_out, _rows(e_br0[ho][:]), _rows(et[:]))

            # ---------------- phase 2: membrane loop -------------------
            QB = 4                      # sign-batch width
            QR = 8                      # mem ring slots (2 batches of slack)
            with tc.tile_pool(name="st", bufs=1) as sp, \
                 tc.tile_pool(name="psw", bufs=3, space="PSUM") as pw:
                minit = sp.tile([128, 128], F32, tag="minit")
                ring = sp.tile([128, QR * 128], F32, tag="ring")
                negone = sp.tile([128, 1], F32, tag="negone")
                nc.vector.memset(negone[:], -1.0)
                nc.sync.dma_start(minit[:], m0_d)
                li4d = li[:].rearrange("p (ho b t) -> p ho b t", ho=HT, b=BC)
                gflat = g[:]
                gw3 = g[:].rearrange("p (ho n) -> p ho n", ho=HT)
                ringq = ring[:].rearrange("p (q f) -> p q f", q=QR)
                for t in range(T):
                    cur = minit[:] if t == 0 else ringq[:, (t - 1) % QR, :]
                    for pr in range(2):
                        nc.vector._custom_dve(
                            lif_op,
                            out=ringq[:, t % QR, 64 * pr:64 * pr + 64].rearrange(
                                "p (ho b) -> p ho b", ho=2),
                            in0=li4d[:, 2 * pr:2 * pr + 2, :, t],
                            in1=cur[:, 64 * pr:64 * pr + 64],
                            s0=alpha[:, 2 * pr:2 * pr + 1],
                            s1=alpha[:, 2 * pr + 1:2 * pr + 2])
                    if t % QB == QB - 1 or t == T - 1:
                        nq = t % QB + 1       # steps in this batch
                        t0 = t - nq + 1
                        # in: ring slots (t0%QR).. as (ho, b, step); out: g
                        # window-major: flat = ho*NSP + c*128 + (b*nq + dt)
                        c = t0 // QB
                        src = dataclasses.replace(
                            ring[:], ap=[ring[:].ap[0], [32, HT], [1, BC],
                                         [128, nq]],
                            offset=ring[:].offset + (t0 % QR) * 128)
                        dst = dataclasses.replace(
                            gflat, ap=[gflat.ap[0], [NSP, HT], [nq, BC],
                                       [1, nq]],
                            offset=gflat.offset + c * 128)
                        nc.scalar.sign(dst, src, bias=negone[:])
                        # classifier for this t-window: samples m = b*nq+dt
                        ptw = pw.tile([128, O], F32, tag="pcls")
                        for ho in range(HT):
                            nc.tensor.matmul(
                                ptw[0:BC * nq, :],
                                gw3[:, ho, c * 128:c * 128 + BC * nq],
                                w2hi[:, ho * O:(ho + 1) * O],
                                start=(ho == 0), stop=False)
                        nc.tensor.matmul(ptw[0:BC * nq, :],
                                         onesr[:, 0:BC * nq], b2p[:, 0:O],
                                         start=False, stop=False)
                        nc.tensor.matmul(ptw[0:BC * nq, :],
                                         onesr[:, 0:BC * nq], b2p[:, O:2 * O],
                                         start=False, stop=True)
                        nc.scalar.copy(
                            logits[0:BC * nq, c * O:(c + 1) * O],
                            ptw[0:BC * nq, :])
                        if BC * nq < 128:
                            nc.vector.memset(
                                logits[BC * nq:128, c * O:(c + 1) * O], 0.0)

            # ---------------- phase 3: head ----------------------------
            with tc.tile_pool(name="cls", bufs=2) as lp, \
                 tc.tile_pool(name="psc", bufs=2, space="PSUM") as pc:
                lg3 = logits[:].rearrange("p (c o) -> p c o", o=O)
                oh3 = oneh[:].rearrange("p (c o) -> p c o", o=O)
                mx = lp.tile([128, NCH], F32, tag="mx")
                nc.vector.tensor_reduce(mx[:], lg3, mybir.AxisListType.X, OP.max)
                ex = lp.tile([128, NCH * O], F32, tag="ex")
                nc.scalar.activation(ex[:].rearrange("p (c o) -> p c o", o=O),
                                     lg3, mybir.ActivationFunctionType.Exp)
                sm = lp.tile([128, NCH], F32, tag="sm")
                nc.vector.tensor_reduce(sm[:],
                                        ex[:].rearrange("p (c o) -> p c o", o=O),
                                        mybir.AxisListType.X, OP.add)
                lse = lp.tile([128, NCH], F32, tag="lse")
                nc.scalar.activation(lse[:], sm[:],
                                     mybir.ActivationFunctionType.Ln)
                tlm = lp.tile([128, NCH * O], F32, tag="tlm")
                nc.vector.tensor_mul(tlm[:].rearrange("p (c o) -> p c o", o=O),
                                     lg3, oh3)
                tl = lp.tile([128, NCH], F32, tag="tl")
                nc.vector.tensor_reduce(tl[:],
                                        tlm[:].rearrange("p (c o) -> p c o", o=O),
                                        mybir.AxisListType.X, OP.add)
                valid = lp.tile([128, NCH], F32, tag="valid")
                nc.vector.tensor_reduce(valid[:], oh3,
                                        mybir.AxisListType.X, OP.add)
                nll = lp.tile([128, NCH], F32, tag="nll")
                nc.vector.tensor_sub(nll[:], lse[:], tl[:])
                nllm = lp.tile([128, NCH], F32, tag="nllm")
                nc.vector.tensor_mul(nllm[:], nll[:], valid[:])
                ind = lp.tile([128, NCH], F32, tag="ind")
                nc.vector.tensor_tensor(ind[:], tl[:], mx[:], OP.is_ge)
                indv = lp.tile([128, NCH], F32, tag="indv")
                nc.vector.tensor_mul(indv[:], ind[:], valid[:])
                fin = lp.tile([128, 2], F32, tag="fin")
                nc.vector.tensor_reduce(fin[:, 0:1], nllm[:],
                                        mybir.AxisListType.X, OP.add)
                nc.vector.tensor_reduce(fin[:, 1:2], indv[:],
                                        mybir.AxisListType.X, OP.add)
                onesc = lp.tile([128, 1], F32, tag="onesc")
                nc.vector.memset(onesc[:], 1.0)
                pfin = pc.tile([1, 2], F32, tag="pfin")
                nc.tensor.matmul(pfin[:], onesc[:], fin[:],
                                 start=True, stop=True)
                outt = lp.tile([1, 2], F32, tag="outt")
                nc.vector.tensor_copy(outt[:], pfin[:])
                nc.sync.dma_start(scal_d, outt[:])

    nc.compile()
    return nc


def _sigmoid(v):
    return 1.0 / (1.0 + np.exp(-v))


def _prep(x, target, mem0, W, tau_m, tau_n, W2, b2, mask):
    x = np.ascontiguousarray(np.asarray(x, np.float32))
    target = np.asarray(target).astype(np.int64)
    mem0 = np.asarray(mem0, np.float32)
    W = np.asarray(W, np.float32)
    tau_m = np.asarray(tau_m, np.float32)
    tau_n = np.asarray(tau_n, np.float32)
    W2 = np.asarray(W2, np.float32)
    b2 = np.asarray(b2, np.float32)
    mask = np.asarray(mask)

    beta = _sigmoid(tau_n).astype(np.float32)          # (H,BR)
    alpha_h = _sigmoid(tau_m).astype(np.float32)       # (H,)

    weff = (W * mask).astype(np.float32)               # (H*BR, D), o = h*2+br
    wre = weff.reshape(H, BR, D).transpose(1, 0, 2)    # (BR,H,D)
    scale = ((1.0 - beta).T * (1.0 - alpha_h)[None, :])
    wfold = (wre * scale[:, :, None]).reshape(H * BR, D)
    wqT = np.zeros((DP, H * BR), np.float32)
    wqT[:D, :] = wfold.T
    wqT = wqT.astype(ml_dtypes.bfloat16)

    beta_r = beta.T.reshape(H * BR)                    # o' = br*512+h
    betab = np.empty((128, MT * BLKZ), np.float32)
    for m in range(MT):
        col = betab[:, m * BLKZ:(m + 1) * BLKZ]
        col[:] = beta_r[m * 128:(m + 1) * 128][:, None]
        col[:, T] = 0.0                                # boundary reset
    alpha = np.empty((128, HT), np.float32)
    for ho in range(HT):
        alpha[:, ho] = alpha_h[ho * 128:(ho + 1) * 128]

    w2s = 0.5 * W2                                      # (O,H)
    w2T = np.empty((128, HT * O), np.float32)
    for ho in range(HT):
        w2T[:, ho * O:(ho + 1) * O] = w2s[:, ho * 128:(ho + 1) * 128].T
    w2hi = w2T.astype(ml_dtypes.bfloat16)

    b2p = (b2 + 0.5 * W2.sum(axis=1)).astype(np.float32)
    b2cat = np.empty((1, 2 * O), np.float32)
    b2hi = b2p.astype(ml_dtypes.bfloat16).astype(np.float32)
    b2cat[0, :O] = b2hi
    b2cat[0, O:] = b2p - b2hi
    b2cat = b2cat.astype(ml_dtypes.bfloat16)

    xt_full = x.transpose(2, 0, 1)                      # (D, B, T)

    in_maps = []
    for c in range(NCORES):
        b0 = c * BC
        xT = np.zeros((DP, NS), np.float32)
        xT[:D, :] = xt_full[:, b0:b0 + BC, :].reshape(D, NS)
        xT = np.ascontiguousarray(xT).astype(ml_dtypes.bfloat16)

        m0 = mem0[b0:b0 + BC]                           # (BC,H)
        m0t = np.ascontiguousarray(
            m0.reshape(BC, HT, 128).transpose(2, 1, 0).reshape(128, 128)
        ).astype(np.float32)

        tgt = target[b0:b0 + BC]                        # (BC, T)
        oneh_f = np.zeros((128, NCH * O), np.float32)
        for c in range(NCH):
            nq = min(4, T - c * 4)
            for dt in range(nq):
                p = np.arange(BC) * nq + dt
                oneh_f[p, c * O + tgt[:, c * 4 + dt]] = 1.0
        oneh = np.ascontiguousarray(oneh_f).astype(ml_dtypes.bfloat16)

        in_maps.append({
            "xT": xT, "wqT": wqT, "betab": betab, "alpha": alpha,
            "m0": m0t, "w2hi": w2hi, "b2p": b2cat, "oneh": oneh,
        })
    return in_maps


def kernel(x, target, mem0, W, tau_m, tau_n, W2, b2, mask):
    if "nc" not in _compiled:
        _compiled["nc"] = _build_nc()
    nc = _compiled["nc"]
    in_maps = _prep(x, target, mem0, W, tau_m, tau_n, W2, b2, mask)
    res = bass_utils.run_bass_kernel_spmd(nc, in_maps,
                                          core_ids=list(range(NCORES)))
    kernel._last_results = res
    loss_sum = 0.0
    corr_sum = 0.0
    for c in range(NCORES):
        s = np.asarray(res.results[c]["scal"], np.float64)
        loss_sum += float(s[0, 0])
        corr_sum += float(s[0, 1])
    loss = np.float32(loss_sum / (B * T))
    correct = np.int32(int(round(corr_sum)))
    return loss, correct, B * T

